# revision 25
# baseline (speedup 1.0000x reference)
"""Multi-head self-attention Trainium2 kernel (8 NeuronCores, head-parallel).

B=2, S=2048, D=1024, H=16, DK=64. Each core owns 2 heads (a 128-wide slice of
the QKV output dims / Wo input dims) and both batch elements. Inputs are
pre-sliced + pre-transposed on the host; partial Wo outputs are summed on the
host (bias added once after the reduce). No cross-core communication.

Device-side layout (all bf16 on PE, fp32 PSUM accumulation):
  xT   [D, T]        activations feature-major (T = B*S tokens)
  QT,KT[128, T]      per-core head dims on partitions
  V    [tok, dk]     token-major, augmented with a ones column per head so the
                     attn @ V matmul also yields softmax denominators
  scores/exp/attn    [k, q] tiles (k on partitions) -> feeds attn @ V directly
  ctxT [128, T]      context feature-major -> feeds Wo projection
  outT [D, T]        per-core partial of the final projection

Schedule: per 512-token q chunk, a k-tile pipeline scores(PE) -> exp(ACT) ->
ctx(PE, dense 8-kt bursts that keep the HAM clock gate warm). Score PSUM
tiles hold k-tile PAIRS so each exp instruction covers 1024 elements/lane.
Normalize chains (softmax denominators broadcast via a DRAM bounce on the
SWDGE rail) run on DVE/DMA overlapped with the next chunk.
"""

import os
import sys

import numpy as np

for _p in ("/opt/trn_rl_repo",):
    if _p not in sys.path and os.path.isdir(_p):
        sys.path.insert(0, _p)

import ml_dtypes  # noqa: E402

import concourse.bass as bass  # noqa: E402
import concourse.mybir as mybir  # noqa: E402
import concourse.tile as tile  # noqa: E402
from concourse import bacc  # noqa: E402

BF16 = ml_dtypes.bfloat16
P = 128

# Problem config (hardcoded per spec)
B, S, D, H = 2, 2048, 1024, 16
DK = D // H            # 64
N_CORES = 8
HPC = H // N_CORES     # heads per core = 2
DSH = HPC * DK         # head dims per core = 128
T = B * S              # 4096 tokens
VW = DSH + HPC         # V width incl. ones cols = 130

TRACE = False
LAST_RESULTS = None


def build_program():
    """Build the single-core Bass program (same program on all 8 cores)."""
    f32 = mybir.dt.float32
    bf16 = mybir.dt.bfloat16
    DT = D // P            # 8 d-tiles (contraction over D)
    KT = S // P            # 16 k-token tiles per batch
    TTI = T // P           # 32 token tiles total
    NOT = D // P           # 8 output tiles for Wo projection
    QW = 512               # q chunk width
    QC = S // QW           # 4 q chunks per batch

    nc = bacc.Bacc(None, target_bir_lowering=False, debug=False)

    # ---- I/O ----
    xt_d = nc.declare_dram_parameter("xt", [D, T], bf16, isOutput=False)
    wq_d = nc.declare_dram_parameter("wq", [D, DSH], bf16, isOutput=False)
    bq_d = nc.declare_dram_parameter("bq", [DSH, 1], f32, isOutput=False)
    wk_d = nc.declare_dram_parameter("wk", [D, DSH], bf16, isOutput=False)
    bk_d = nc.declare_dram_parameter("bk", [DSH, 1], f32, isOutput=False)
    wv_d = nc.declare_dram_parameter("wv", [D, VW], bf16, isOutput=False)
    bv_d = nc.declare_dram_parameter("bv", [1, VW], bf16, isOutput=False)
    wo_d = nc.declare_dram_parameter("wo", [DSH, D], bf16, isOutput=False)
    attn_d = nc.declare_dram_parameter("attn", [B, HPC, S, S], bf16, isOutput=True)
    out_d = nc.declare_dram_parameter("out", [D, T], bf16, isOutput=True)

    AF = mybir.ActivationFunctionType
    OP = mybir.AluOpType

    with tile.TileContext(nc) as tc:
        with (
            tc.tile_pool(name="const", bufs=1) as cpool,
            tc.tile_pool(name="big", bufs=1) as bpool,
        ):
            # persistent SBUF tensors
            wq_sb = cpool.tile([P, DT, DSH], bf16, tag="wq")
            wk_sb = cpool.tile([P, DT, DSH], bf16, tag="wk")
            wv_sb = cpool.tile([P, DT, VW], bf16, tag="wv")
            wo_sb = cpool.tile([P, D], bf16, tag="wo")
            bq_sb = cpool.tile([P, 1], f32, tag="bq")
            bk_sb = cpool.tile([P, 1], f32, tag="bk")
            bvb_sb = cpool.tile([P, VW], bf16, tag="bvb")

            qt_sb = bpool.tile([P, T], bf16, tag="qt")
            kt_sb = bpool.tile([P, T], bf16, tag="kt")
            v_sb = bpool.tile([P, TTI, VW], bf16, tag="v")
            ctxt_sb = bpool.tile([P, T], bf16, tag="ctxt")

            nc.sync.dma_start(wq_sb[:], wq_d.rearrange("(dt p) m -> p dt m", p=P))
            nc.sync.dma_start(wk_sb[:], wk_d.rearrange("(dt p) m -> p dt m", p=P))
            nc.sync.dma_start(bq_sb[:], bq_d[:])
            nc.sync.dma_start(bk_sb[:], bk_d[:])

            # ACT warmup: force the exp table-set load onto a wait-free inst
            warm_sb = cpool.tile([1, 8], f32, tag="warm")
            nc.vector.memset(warm_sb[:], 0.0)
            nc.scalar.activation(warm_sb[:], warm_sb[:], AF.Exp)

            with (
                tc.tile_pool(name="xt", bufs=1) as xpool,
                tc.tile_pool(name="exp", bufs=2) as epool,
                tc.tile_pool(name="bc", bufs=1) as bcpool,
                tc.tile_pool(name="ost", bufs=8) as opool,
                tc.tile_pool(name="dbc", bufs=2, space="DRAM") as dpool,
                tc.tile_pool(name="psS", bufs=1, space="PSUM") as psS,
                tc.tile_pool(name="psC", bufs=1, space="PSUM") as psC,
                tc.tile_pool(name="psX", bufs=1, space="PSUM") as psX,
            ):
                # ---------------- Phase A: projections ----------------
                xt_r = xt_d.rearrange("(dt p) t -> p dt t", p=P)
                xt_tiles = []
                for ch in range(T // 512):
                    xt_c = xpool.tile(
                        [P, DT, 512], bf16, tag=f"xt{ch}", name=f"xt{ch}"
                    )
                    nc.sync.dma_start(
                        xt_c[:], xt_r[:, :, ch * 512 : (ch + 1) * 512]
                    )
                    xt_tiles.append(xt_c)

                def qk_proj(ch):
                    # one 512-token chunk of the Q and K projections
                    sl = slice(ch * 512, (ch + 1) * 512)
                    for w_sb, b_sb, dst, ptag in (
                        (wq_sb, bq_sb, qt_sb, "psqk"),
                        (wk_sb, bk_sb, kt_sb, "psv"),
                    ):
                        ps = psX.tile([P, 512], f32, tag=ptag, name="psqk")
                        for dt in range(DT):
                            nc.tensor.matmul(
                                ps[:],
                                lhsT=w_sb[:, dt, :],
                                rhs=xt_tiles[ch][:, dt, :],
                                start=(dt == 0),
                                stop=(dt == DT - 1),
                            )
                        nc.scalar.activation(
                            dst[:, sl], ps[:], AF.Identity, bias=b_sb[:]
                        )

                def v_proj(tt):
                    # one 128-token tile of the [V | 1] projection
                    psv = psX.tile([P, 512], f32, tag="psv", name="psv")[:, :VW]
                    for dt in range(DT):
                        nc.tensor.matmul(
                            psv[:],
                            lhsT=xt_tiles[tt // 4][
                                :, dt, (tt % 4) * P : (tt % 4 + 1) * P
                            ],
                            rhs=wv_sb[:, dt, :],
                            start=(dt == 0),
                            stop=(dt == DT - 1),
                        )
                    nc.vector.tensor_tensor(
                        v_sb[:, tt, :], psv[:], bvb_sb[:], OP.add
                    )

                # phase A covers batch 0 only; batch 1 projections are
                # interleaved into batch 0's attention loop as PE filler
                for ch in range(S // 512):
                    qk_proj(ch)
                nc.sync.dma_start(
                    wv_sb[:], wv_d.rearrange("(dt p) m -> p dt m", p=P)
                )
                nc.sync.dma_start(wo_sb[:], wo_d[:])
                nc.sync.dma_start(bvb_sb[:], bv_d[:].to_broadcast((P, VW)))
                for tt in range(TTI // 2):
                    v_proj(tt)
                fill_units = [("qk", ch) for ch in range(S // 512, T // 512)] + [
                    ("v", tt) for tt in range(TTI // 2, TTI)
                ]

                # ---------------- Phase B: attention ----------------
                for b in range(B):
                    for qc in range(QC):
                        q0 = b * S + qc * QW  # token offset of q chunk
                        qsl = slice(q0, q0 + QW)
                        exps = [
                            epool.tile(
                                [P, KT, QW], bf16, tag=f"exp{hl}", name=f"exp{hl}"
                            )
                            for hl in range(HPC)
                        ]
                        # ctx accumulators (row DK = softmax denominators via
                        # the ones column of V)
                        cacc = [
                            psC.tile([P, QW], f32, tag=f"C{hl}", name=f"C{hl}")[
                                : DK + 1, :
                            ]
                            for hl in range(HPC)
                        ]
                        # k-tile pipeline over pairs: scores for kt, kt+1 land
                        # in one [P, 2, QW] PSUM tile per head; one exp op per
                        # pair per head (1024 elem/lane amortizes ACT setup).
                        # A/B head ping-pong double-buffers the score tiles.
                        for kp in range(KT // 2):
                            pss = [
                                psS.tile(
                                    [P, 2, QW], f32, tag=f"S{hl}", name=f"S{hl}"
                                )
                                for hl in range(HPC)
                            ]
                            for k2 in range(2):
                                kt = 2 * kp + k2
                                ksl = slice(b * S + kt * P, b * S + (kt + 1) * P)
                                for hl in range(HPC):
                                    hp = slice(hl * DK, (hl + 1) * DK)
                                    nc.tensor.matmul(
                                        pss[hl][:, k2, :],
                                        lhsT=kt_sb[hp, ksl],
                                        rhs=qt_sb[hp, qsl],
                                        start=True,
                                        stop=True,
                                        tile_position=(hl * DK, 0),
                                    )
                            for hl in range(HPC):
                                nc.scalar.activation(
                                    exps[hl][:, 2 * kp : 2 * kp + 2, :],
                                    pss[hl][:],
                                    AF.Exp,
                                )
                            # dense ctx bursts re-warm the clock gate; the
                            # last burst is kept tiny so the next chunk's
                            # scores (and the ACT exp stream) aren't delayed
                            def ctx_burst(k_lo, k_hi):
                                for kk in range(k_lo, k_hi):
                                    vtk = b * KT + kk
                                    for hl in range(HPC):
                                        vsl = slice(
                                            hl * (DK + 1), (hl + 1) * (DK + 1)
                                        )
                                        nc.tensor.matmul(
                                            cacc[hl][:],
                                            lhsT=v_sb[:, vtk, vsl],
                                            rhs=exps[hl][:, kk, :],
                                            start=(kk == 0),
                                            stop=(kk == KT - 1),
                                        )

                            if kp == 3:
                                ctx_burst(0, 8)
                            elif kp == 6:
                                ctx_burst(8, 14)
                            elif kp == 7:
                                ctx_burst(14, 16)
                            elif b == 0 and fill_units:
                                kind, arg = fill_units.pop(0)
                                if kind == "qk":
                                    qk_proj(arg)
                                else:
                                    v_proj(arg)
                        # Stage ctx+sums out of PSUM immediately so the C
                        # slots free up without waiting the normalize chain.
                        stages = []
                        for hl in range(HPC):
                            st = bcpool.tile(
                                [DK + 1, QW], f32, tag=f"st{hl}", name=f"st{hl}"
                            )
                            nc.vector.tensor_copy(out=st[:], in_=cacc[hl][:])
                            stages.append(st)
                        # normalize + writeout (overlaps the next chunk)
                        for hl in range(HPC):
                            bcf = bcpool.tile([P, QW], f32, tag="bcf", name="bcf")
                            bc16 = bcpool.tile(
                                [P, QW], bf16, tag=f"bc16{hl}", name=f"bc16{hl}"
                            )
                            sums_dr = dpool.tile(
                                [1, QW], f32, tag=f"sd{hl}", name=f"sd{hl}"
                            )
                            nc.gpsimd.dma_start(
                                sums_dr[:], stages[hl][DK : DK + 1, :]
                            )
                            nc.gpsimd.dma_start(
                                bcf[:], sums_dr[:].to_broadcast((P, QW))
                            )
                            nc.vector.reciprocal_approx_fast(out=bcf[:], in_=bcf[:])
                            nc.vector.tensor_copy(out=bc16[:], in_=bcf[:])
                            # ctxT = unnorm_ctx * (1/sum)
                            nc.vector.tensor_tensor(
                                ctxt_sb[hl * DK : (hl + 1) * DK, qsl],
                                stages[hl][0:DK, :],
                                bc16[0:DK, :],
                                OP.mult,
                            )
                            # attn = exp * (1/sum) in place, then DMA out in
                            # quarters so the HWDGE rail drains early
                            attn_r = attn_d[b, hl, :, qc * QW : (qc + 1) * QW]
                            attn_r = attn_r.rearrange("(kt p) q -> p kt q", p=P)
                            for nq in range(4):
                                kq = slice(nq * KT // 4, (nq + 1) * KT // 4)
                                nc.vector.tensor_tensor(
                                    exps[hl][:, kq, :],
                                    exps[hl][:, kq, :],
                                    bc16[:, None, :].to_broadcast(
                                        (P, KT // 4, QW)
                                    ),
                                    OP.mult,
                                )
                                nc.sync.dma_start(
                                    attn_r[:, kq, :], exps[hl][:, kq, :]
                                )

                # ---------- Phase C: output projection ----------
                # All engines/banks free: 6-way PSUM rotation, 8-deep staging,
                # ACT handles the first half of the copies (DVE still drains
                # the last chunks' normalize work), out DMAs on SWDGE.
                ptags = [
                    ("S0", psS),
                    ("S1", psS),
                    ("C0", psC),
                    ("C1", psC),
                    ("psqk", psX),
                    ("psv", psX),
                ]
                j = 0
                for ch in range(T // 512):
                    sl = slice(ch * 512, (ch + 1) * 512)
                    for ot in range(NOT):
                        osl = slice(ot * P, (ot + 1) * P)
                        ptag, pool = ptags[j % 6]
                        pso = pool.tile([P, 512], f32, tag=ptag, name=f"pso{j}")
                        nc.tensor.matmul(
                            pso[:],
                            lhsT=wo_sb[:, osl],
                            rhs=ctxt_sb[:, sl],
                            start=True,
                            stop=True,
                        )
                        ou = opool.tile([P, 512], bf16, tag="ou", name="ou")
                        if j % 2 == 0:
                            nc.scalar.copy(ou[:], pso[:])
                        else:
                            nc.vector.tensor_copy(out=ou[:], in_=pso[:])
                        nc.gpsimd.dma_start(out_d[osl, sl], ou[:])
                        j += 1

    nc.compile()
    return nc


_NC_CACHE = {}


def _get_program():
    if "nc" not in _NC_CACHE:
        _NC_CACHE["nc"] = build_program()
    return _NC_CACHE["nc"]


def _prep_inputs(x, Wq, bq, Wk, bk, Wv, bv, Wo, bo):
    """Host-side sharding + layout prep. Returns in_maps (one dict per core)."""
    scale = 1.0 / np.sqrt(np.float32(DK))  # folded into Wq/bq
    xt = np.ascontiguousarray(x.reshape(T, D).T).astype(BF16)
    in_maps = []
    for c in range(N_CORES):
        sl = slice(DSH * c, DSH * (c + 1))
        wq_c = np.ascontiguousarray((Wq[sl] * scale).T).astype(BF16)
        bq_c = (bq[sl] * scale).astype(np.float32)[:, None]
        wk_c = np.ascontiguousarray(Wk[sl].T).astype(BF16)
        bk_c = bk[sl].astype(np.float32)[:, None]
        wv_c = np.zeros((D, VW), np.float32)
        bv_c = np.zeros((1, VW), np.float32)
        for hl in range(HPC):
            rows = slice(DSH * c + hl * DK, DSH * c + (hl + 1) * DK)
            cols = slice(hl * (DK + 1), hl * (DK + 1) + DK)
            wv_c[:, cols] = Wv[rows].T
            bv_c[0, cols] = bv[rows]
            bv_c[0, hl * (DK + 1) + DK] = 1.0
        wo_c = np.ascontiguousarray(Wo[:, sl].T).astype(BF16)
        in_maps.append(
            {
                "xt": xt,
                "wq": wq_c,
                "bq": bq_c,
                "wk": wk_c,
                "bk": bk_c,
                "wv": wv_c.astype(BF16),
                "bv": bv_c.astype(BF16),
                "wo": wo_c,
            }
        )
    return in_maps


def kernel(x, Wq, bq, Wk, bk, Wv, bv, Wo, bo):
    global LAST_RESULTS
    from concourse.bass_utils import run_bass_kernel_spmd

    x = np.asarray(x, np.float32)
    in_maps = _prep_inputs(
        x,
        np.asarray(Wq, np.float32),
        np.asarray(bq, np.float32),
        np.asarray(Wk, np.float32),
        np.asarray(bk, np.float32),
        np.asarray(Wv, np.float32),
        np.asarray(bv, np.float32),
        np.asarray(Wo, np.float32),
        np.asarray(bo, np.float32),
    )
    nc = _get_program()
    res = run_bass_kernel_spmd(nc, in_maps, list(range(N_CORES)), trace=TRACE)
    LAST_RESULTS = res

    # ---- gather / unshard ----
    out_t = np.zeros((D, T), np.float32)
    out_t += np.asarray(bo, np.float32).reshape(1, D).T  # bias once, post-reduce
    attn = np.empty((B, H, S, S), np.float32)
    for c in range(N_CORES):
        r = res.results[c]
        out_t += np.asarray(r["out"]).astype(np.float32)
        a = np.asarray(r["attn"])  # [B, HPC, S(k), S(q)] bf16
        for hl in range(HPC):
            attn[:, HPC * c + hl] = np.swapaxes(
                a[:, hl].astype(np.float32), 1, 2
            )
    output = np.ascontiguousarray(out_t.T).reshape(B, S, D)
    return output, attn


# revision 27
# speedup vs baseline: 1.0479x; 1.0479x over previous
"""Multi-head self-attention Trainium2 kernel (8 NeuronCores, head-parallel).

B=2, S=2048, D=1024, H=16, DK=64. Each core owns 2 heads (a 128-wide slice of
the QKV output dims / Wo input dims) and both batch elements. Inputs are
pre-sliced + pre-transposed on the host; partial Wo outputs are summed on the
host (bias added once after the reduce). No cross-core communication.

Device-side layout (all bf16 on PE, fp32 PSUM accumulation):
  xT   [D, T]        activations feature-major (T = B*S tokens)
  QT,KT[128, T]      per-core head dims on partitions
  V    [tok, dk]     token-major, augmented with a ones column per head so the
                     attn @ V matmul also yields softmax denominators
  scores/exp/attn    [k, q] tiles (k on partitions) -> feeds attn @ V directly
  ctxT [128, T]      context feature-major -> feeds Wo projection
  outT [D, T]        per-core partial of the final projection

Schedule: per 512-token q chunk, a k-tile pipeline scores(PE) -> exp(ACT) ->
ctx(PE, dense 8-kt bursts that keep the HAM clock gate warm). Score PSUM
tiles hold k-tile PAIRS so each exp instruction covers 1024 elements/lane.
Normalize chains (softmax denominators broadcast via a DRAM bounce on the
SWDGE rail) run on DVE/DMA overlapped with the next chunk.
"""

import os
import sys

import numpy as np

for _p in ("/opt/trn_rl_repo",):
    if _p not in sys.path and os.path.isdir(_p):
        sys.path.insert(0, _p)

import ml_dtypes  # noqa: E402

import concourse.bass as bass  # noqa: E402
import concourse.mybir as mybir  # noqa: E402
import concourse.tile as tile  # noqa: E402
from concourse import bacc  # noqa: E402

BF16 = ml_dtypes.bfloat16
P = 128

# Problem config (hardcoded per spec)
B, S, D, H = 2, 2048, 1024, 16
DK = D // H            # 64
N_CORES = 8
HPC = H // N_CORES     # heads per core = 2
DSH = HPC * DK         # head dims per core = 128
T = B * S              # 4096 tokens
VW = DSH + HPC         # V width incl. ones cols = 130

TRACE = False
LAST_RESULTS = None


def build_program():
    """Build the single-core Bass program (same program on all 8 cores)."""
    f32 = mybir.dt.float32
    bf16 = mybir.dt.bfloat16
    DT = D // P            # 8 d-tiles (contraction over D)
    KT = S // P            # 16 k-token tiles per batch
    TTI = T // P           # 32 token tiles total
    NOT = D // P           # 8 output tiles for Wo projection
    QW = 512               # q chunk width
    QC = S // QW           # 4 q chunks per batch

    nc = bacc.Bacc(None, target_bir_lowering=False, debug=False)

    # ---- I/O ----
    xt_d = nc.declare_dram_parameter("xt", [D, T], bf16, isOutput=False)
    wq_d = nc.declare_dram_parameter("wq", [D, DSH], bf16, isOutput=False)
    bq_d = nc.declare_dram_parameter("bq", [DSH, 1], f32, isOutput=False)
    wk_d = nc.declare_dram_parameter("wk", [D, DSH], bf16, isOutput=False)
    bk_d = nc.declare_dram_parameter("bk", [DSH, 1], f32, isOutput=False)
    wv_d = nc.declare_dram_parameter("wv", [D, VW], bf16, isOutput=False)
    bv_d = nc.declare_dram_parameter("bv", [1, VW], bf16, isOutput=False)
    wo_d = nc.declare_dram_parameter("wo", [DSH, D], bf16, isOutput=False)
    attn_d = nc.declare_dram_parameter("attn", [B, HPC, S, S], bf16, isOutput=True)
    out_d = nc.declare_dram_parameter("out", [D, T], bf16, isOutput=True)

    AF = mybir.ActivationFunctionType
    OP = mybir.AluOpType

    with tile.TileContext(nc) as tc:
        with (
            tc.tile_pool(name="const", bufs=1) as cpool,
            tc.tile_pool(name="big", bufs=1) as bpool,
        ):
            # persistent SBUF tensors
            wq_sb = cpool.tile([P, DT, DSH], bf16, tag="wq")
            wk_sb = cpool.tile([P, DT, DSH], bf16, tag="wk")
            wv_sb = cpool.tile([P, DT, VW], bf16, tag="wv")
            wo_sb = cpool.tile([P, D], bf16, tag="wo")
            bq_sb = cpool.tile([P, 1], f32, tag="bq")
            bk_sb = cpool.tile([P, 1], f32, tag="bk")
            bvb_sb = cpool.tile([P, VW], bf16, tag="bvb")

            qt_sb = bpool.tile([P, T], bf16, tag="qt")
            kt_sb = bpool.tile([P, T], bf16, tag="kt")
            v_sb = bpool.tile([P, TTI, VW], bf16, tag="v")
            ctxt_sb = bpool.tile([P, T], bf16, tag="ctxt")

            nc.sync.dma_start(wq_sb[:], wq_d.rearrange("(dt p) m -> p dt m", p=P))
            nc.sync.dma_start(wk_sb[:], wk_d.rearrange("(dt p) m -> p dt m", p=P))
            nc.sync.dma_start(bq_sb[:], bq_d[:])
            nc.sync.dma_start(bk_sb[:], bk_d[:])

            # ACT warmup: force the exp table-set load onto a wait-free inst
            warm_sb = cpool.tile([1, 8], f32, tag="warm")
            nc.vector.memset(warm_sb[:], 0.0)
            nc.scalar.activation(warm_sb[:], warm_sb[:], AF.Exp)

            with (
                tc.tile_pool(name="xt", bufs=1) as xpool,
                tc.tile_pool(name="exp", bufs=2) as epool,
                tc.tile_pool(name="bc", bufs=1) as bcpool,
                tc.tile_pool(name="ost", bufs=8) as opool,
                tc.tile_pool(name="dbc", bufs=2, space="DRAM") as dpool,
                tc.tile_pool(name="psS", bufs=1, space="PSUM") as psS,
                tc.tile_pool(name="psC", bufs=1, space="PSUM") as psC,
                tc.tile_pool(name="psX", bufs=1, space="PSUM") as psX,
            ):
                # ---------------- Phase A: projections ----------------
                xt_r = xt_d.rearrange("(dt p) t -> p dt t", p=P)
                xt_tiles = []
                for ch in range(T // 512):
                    xt_c = xpool.tile(
                        [P, DT, 512], bf16, tag=f"xt{ch}", name=f"xt{ch}"
                    )
                    nc.sync.dma_start(
                        xt_c[:], xt_r[:, :, ch * 512 : (ch + 1) * 512]
                    )
                    xt_tiles.append(xt_c)

                def qk_proj(ch, which, on_act=True):
                    # one 512-token chunk of the Q or K projection
                    sl = slice(ch * 512, (ch + 1) * 512)
                    w_sb, b_sb, dst, ptag = (
                        (wq_sb, bq_sb, qt_sb, "psqk")
                        if which == "q"
                        else (wk_sb, bk_sb, kt_sb, "psv")
                    )
                    ps = psX.tile([P, 512], f32, tag=ptag, name="psqk")
                    for dt in range(DT):
                        nc.tensor.matmul(
                            ps[:],
                            lhsT=w_sb[:, dt, :],
                            rhs=xt_tiles[ch][:, dt, :],
                            start=(dt == 0),
                            stop=(dt == DT - 1),
                        )
                    if on_act:
                        nc.scalar.activation(
                            dst[:, sl], ps[:], AF.Identity, bias=b_sb[:]
                        )
                    else:
                        nc.vector.tensor_tensor(
                            dst[:, sl],
                            ps[:],
                            b_sb[:, 0:1].to_broadcast((P, 512)),
                            OP.add,
                        )

                def v_proj(tt):
                    # one 128-token tile of the [V | 1] projection
                    psv = psX.tile([P, 512], f32, tag="psv", name="psv")[:, :VW]
                    for dt in range(DT):
                        nc.tensor.matmul(
                            psv[:],
                            lhsT=xt_tiles[tt // 4][
                                :, dt, (tt % 4) * P : (tt % 4 + 1) * P
                            ],
                            rhs=wv_sb[:, dt, :],
                            start=(dt == 0),
                            stop=(dt == DT - 1),
                        )
                    nc.vector.tensor_tensor(
                        v_sb[:, tt, :], psv[:], bvb_sb[:], OP.add
                    )

                def outproj_unit(ch):
                    # Wo projection for one 512-token chunk; PSUM borrowed
                    # from the psX tags (free outside phase A / proj units),
                    # copies on DVE, out DMA on the SWDGE rail
                    sl = slice(ch * 512, (ch + 1) * 512)
                    for ot in range(NOT):
                        osl = slice(ot * P, (ot + 1) * P)
                        pso = psX.tile(
                            [P, 512],
                            f32,
                            tag="psqk" if ot % 2 == 0 else "psv",
                            name=f"pso{ch}_{ot}",
                        )
                        nc.tensor.matmul(
                            pso[:],
                            lhsT=wo_sb[:, osl],
                            rhs=ctxt_sb[:, sl],
                            start=True,
                            stop=True,
                        )
                        ou = opool.tile([P, 512], bf16, tag="ou", name="ou")
                        nc.vector.tensor_copy(out=ou[:], in_=pso[:])
                        nc.gpsimd.dma_start(out_d[osl, sl], ou[:])

                # phase A covers batch 0 only; batch 1 projections are
                # interleaved into batch 0's attention loop as PE filler,
                # and each chunk's Wo projection into the following chunks
                for ch in range(S // 512):
                    qk_proj(ch, "q")
                    qk_proj(ch, "k")
                nc.sync.dma_start(
                    wv_sb[:], wv_d.rearrange("(dt p) m -> p dt m", p=P)
                )
                nc.sync.dma_start(wo_sb[:], wo_d[:])
                nc.sync.dma_start(bvb_sb[:], bv_d[:].to_broadcast((P, VW)))
                for tt in range(TTI // 2 + 4):
                    v_proj(tt)
                fill_units = []
                for ch in range(S // 512, T // 512):
                    fill_units.append(("q", ch))
                    fill_units.append(("k", ch))
                for tt in range(TTI // 2 + 4, TTI):
                    fill_units.append(("v", tt))
                # out-projection schedule: chunk ii-4 handled inside
                # instance ii (b=1); chunks 4..7 at the tail
                out_sched = {4: [0], 5: [1, 2], 6: [3, 4], 7: [5, 6]}

                # ---------------- Phase B: attention ----------------
                for b in range(B):
                    for qc in range(QC):
                        ii = b * QC + qc
                        out_pending = list(out_sched.get(ii, []))
                        q0 = b * S + qc * QW  # token offset of q chunk
                        qsl = slice(q0, q0 + QW)
                        exps = [
                            epool.tile(
                                [P, KT, QW], bf16, tag=f"exp{hl}", name=f"exp{hl}"
                            )
                            for hl in range(HPC)
                        ]
                        # ctx accumulators (row DK = softmax denominators via
                        # the ones column of V)
                        cacc = [
                            psC.tile([P, QW], f32, tag=f"C{hl}", name=f"C{hl}")[
                                : DK + 1, :
                            ]
                            for hl in range(HPC)
                        ]
                        # k-tile pipeline over pairs: scores for kt, kt+1 land
                        # in one [P, 2, QW] PSUM tile per head; one exp op per
                        # pair per head (1024 elem/lane amortizes ACT setup).
                        # A/B head ping-pong double-buffers the score tiles.
                        for kp in range(KT // 2):
                            pss = [
                                psS.tile(
                                    [P, 2, QW], f32, tag=f"S{hl}", name=f"S{hl}"
                                )
                                for hl in range(HPC)
                            ]
                            for k2 in range(2):
                                kt = 2 * kp + k2
                                ksl = slice(b * S + kt * P, b * S + (kt + 1) * P)
                                for hl in range(HPC):
                                    hp = slice(hl * DK, (hl + 1) * DK)
                                    nc.tensor.matmul(
                                        pss[hl][:, k2, :],
                                        lhsT=kt_sb[hp, ksl],
                                        rhs=qt_sb[hp, qsl],
                                        start=True,
                                        stop=True,
                                        tile_position=(hl * DK, 0),
                                    )
                            for hl in range(HPC):
                                nc.scalar.activation(
                                    exps[hl][:, 2 * kp : 2 * kp + 2, :],
                                    pss[hl][:],
                                    AF.Exp,
                                )
                            # dense ctx bursts re-warm the clock gate; the
                            # last burst is kept tiny so the next chunk's
                            # scores (and the ACT exp stream) aren't delayed
                            def ctx_burst(k_lo, k_hi):
                                for kk in range(k_lo, k_hi):
                                    vtk = b * KT + kk
                                    for hl in range(HPC):
                                        vsl = slice(
                                            hl * (DK + 1), (hl + 1) * (DK + 1)
                                        )
                                        nc.tensor.matmul(
                                            cacc[hl][:],
                                            lhsT=v_sb[:, vtk, vsl],
                                            rhs=exps[hl][:, kk, :],
                                            start=(kk == 0),
                                            stop=(kk == KT - 1),
                                        )

                            if kp == 3:
                                ctx_burst(0, 8)
                            elif kp == 6:
                                ctx_burst(8, 14)
                            elif kp == 7:
                                ctx_burst(14, 16)
                            elif b == 0 and fill_units:
                                kind, arg = fill_units.pop(0)
                                if kind == "v":
                                    v_proj(arg)
                                else:
                                    qk_proj(arg, kind, on_act=False)
                            elif b == 1 and out_pending:
                                outproj_unit(out_pending.pop(0))
                        # Stage ctx+sums out of PSUM immediately so the C
                        # slots free up without waiting the normalize chain.
                        stages = []
                        for hl in range(HPC):
                            st = bcpool.tile(
                                [DK + 1, QW], f32, tag=f"st{hl}", name=f"st{hl}"
                            )
                            nc.vector.tensor_copy(out=st[:], in_=cacc[hl][:])
                            stages.append(st)
                        # normalize + writeout (overlaps the next chunk)
                        for hl in range(HPC):
                            bcf = bcpool.tile([P, QW], f32, tag="bcf", name="bcf")
                            bc16 = bcpool.tile(
                                [P, QW], bf16, tag=f"bc16{hl}", name=f"bc16{hl}"
                            )
                            sums_dr = dpool.tile(
                                [1, QW], f32, tag=f"sd{hl}", name=f"sd{hl}"
                            )
                            nc.gpsimd.dma_start(
                                sums_dr[:], stages[hl][DK : DK + 1, :]
                            )
                            nc.gpsimd.dma_start(
                                bcf[:], sums_dr[:].to_broadcast((P, QW))
                            )
                            nc.vector.reciprocal_approx_fast(out=bcf[:], in_=bcf[:])
                            nc.vector.tensor_copy(out=bc16[:], in_=bcf[:])
                            # ctxT = unnorm_ctx * (1/sum)
                            nc.vector.tensor_tensor(
                                ctxt_sb[hl * DK : (hl + 1) * DK, qsl],
                                stages[hl][0:DK, :],
                                bc16[0:DK, :],
                                OP.mult,
                            )
                            # attn = exp * (1/sum) in place, then DMA out in
                            # quarters so the HWDGE rail drains early
                            attn_r = attn_d[b, hl, :, qc * QW : (qc + 1) * QW]
                            attn_r = attn_r.rearrange("(kt p) q -> p kt q", p=P)
                            for nq in range(4):
                                kq = slice(nq * KT // 4, (nq + 1) * KT // 4)
                                nc.vector.tensor_tensor(
                                    exps[hl][:, kq, :],
                                    exps[hl][:, kq, :],
                                    bc16[:, None, :].to_broadcast(
                                        (P, KT // 4, QW)
                                    ),
                                    OP.mult,
                                )
                                nc.sync.dma_start(
                                    attn_r[:, kq, :], exps[hl][:, kq, :]
                                )

                # ---------- tail: remaining Wo chunk ----------
                # Only the last chunk is left; PSUM/engines are free.
                # Copies on ACT (DVE drains the final normalize chain).
                ptags = [("S0", psS), ("S1", psS), ("C0", psC), ("C1", psC)]
                j = 0
                for ch in [7]:
                    sl = slice(ch * 512, (ch + 1) * 512)
                    for ot in range(NOT):
                        osl = slice(ot * P, (ot + 1) * P)
                        ptag, pool = ptags[j % 4]
                        pso = pool.tile([P, 512], f32, tag=ptag, name=f"psoT{j}")
                        nc.tensor.matmul(
                            pso[:],
                            lhsT=wo_sb[:, osl],
                            rhs=ctxt_sb[:, sl],
                            start=True,
                            stop=True,
                        )
                        ou = opool.tile([P, 512], bf16, tag="ou", name="ou")
                        nc.scalar.copy(ou[:], pso[:])
                        nc.gpsimd.dma_start(out_d[osl, sl], ou[:])
                        j += 1

    nc.compile()
    return nc


_NC_CACHE = {}


def _get_program():
    if "nc" not in _NC_CACHE:
        _NC_CACHE["nc"] = build_program()
    return _NC_CACHE["nc"]


def _prep_inputs(x, Wq, bq, Wk, bk, Wv, bv, Wo, bo):
    """Host-side sharding + layout prep. Returns in_maps (one dict per core)."""
    scale = 1.0 / np.sqrt(np.float32(DK))  # folded into Wq/bq
    xt = np.ascontiguousarray(x.reshape(T, D).T).astype(BF16)
    in_maps = []
    for c in range(N_CORES):
        sl = slice(DSH * c, DSH * (c + 1))
        wq_c = np.ascontiguousarray((Wq[sl] * scale).T).astype(BF16)
        bq_c = (bq[sl] * scale).astype(np.float32)[:, None]
        wk_c = np.ascontiguousarray(Wk[sl].T).astype(BF16)
        bk_c = bk[sl].astype(np.float32)[:, None]
        wv_c = np.zeros((D, VW), np.float32)
        bv_c = np.zeros((1, VW), np.float32)
        for hl in range(HPC):
            rows = slice(DSH * c + hl * DK, DSH * c + (hl + 1) * DK)
            cols = slice(hl * (DK + 1), hl * (DK + 1) + DK)
            wv_c[:, cols] = Wv[rows].T
            bv_c[0, cols] = bv[rows]
            bv_c[0, hl * (DK + 1) + DK] = 1.0
        wo_c = np.ascontiguousarray(Wo[:, sl].T).astype(BF16)
        in_maps.append(
            {
                "xt": xt,
                "wq": wq_c,
                "bq": bq_c,
                "wk": wk_c,
                "bk": bk_c,
                "wv": wv_c.astype(BF16),
                "bv": bv_c.astype(BF16),
                "wo": wo_c,
            }
        )
    return in_maps


def kernel(x, Wq, bq, Wk, bk, Wv, bv, Wo, bo):
    global LAST_RESULTS
    from concourse.bass_utils import run_bass_kernel_spmd

    x = np.asarray(x, np.float32)
    in_maps = _prep_inputs(
        x,
        np.asarray(Wq, np.float32),
        np.asarray(bq, np.float32),
        np.asarray(Wk, np.float32),
        np.asarray(bk, np.float32),
        np.asarray(Wv, np.float32),
        np.asarray(bv, np.float32),
        np.asarray(Wo, np.float32),
        np.asarray(bo, np.float32),
    )
    nc = _get_program()
    res = run_bass_kernel_spmd(nc, in_maps, list(range(N_CORES)), trace=TRACE)
    LAST_RESULTS = res

    # ---- gather / unshard ----
    out_t = np.zeros((D, T), np.float32)
    out_t += np.asarray(bo, np.float32).reshape(1, D).T  # bias once, post-reduce
    attn = np.empty((B, H, S, S), np.float32)
    for c in range(N_CORES):
        r = res.results[c]
        out_t += np.asarray(r["out"]).astype(np.float32)
        a = np.asarray(r["attn"])  # [B, HPC, S(k), S(q)] bf16
        for hl in range(HPC):
            attn[:, HPC * c + hl] = np.swapaxes(
                a[:, hl].astype(np.float32), 1, 2
            )
    output = np.ascontiguousarray(out_t.T).reshape(B, S, D)
    return output, attn
